# revision 6
# baseline (speedup 1.0000x reference)
"""Fused cross-attention Bass/Tile kernel for Trainium2, batch-sharded over 8 cores.

Per core (one batch element):
  Q^T = Wq @ x^T + bq      [D, NQ]   (e on partitions)
  K^T = Wk @ ctx^T + bk    [D, NK]
  V   = ctx @ Wv^T         [NK, D]   (bv deferred to the output)
  S^T = K^T.T-contraction: S^T[m, n] = sum_e K^T[e,m] Q^T[e,n]   (PE, f32r)
  E^T = exp(scale * S^T)   (ACT, PSUM->SBUF)
  O   += E^T.T @ V         (PE accumulation over m-tiles)
  eacc += E^T              (Pool engine, SBUF accumulator)
  rs   = reduce(eacc.T)    (PE transpose + DVE free-dim reduce, per q-chunk)
  out = O / rs + bv        (DVE scalar_tensor_tensor)

The S^T orientation means softmax normalization needs no P-transpose and the
PV matmul consumes E^T directly as the stationary operand.  Row sums are
accumulated on the (otherwise idle) Pool engine instead of N=1 PE matmuls,
which saves 512 PE cycles per m-tile (~51 us/core).
"""

import contextlib
import sys

if "/opt/trn_rl_repo" not in sys.path:
    sys.path.insert(0, "/opt/trn_rl_repo")

import numpy as np

import concourse.bass as bass
import concourse.mybir as mybir
import concourse.tile as tile
from concourse.bass_utils import run_bass_kernel_spmd
from concourse.masks import make_identity

P = 128
N_CORES = 8
F32 = mybir.dt.float32
F32R = mybir.dt.float32r
BF16 = mybir.dt.bfloat16


def _split_drain_waits(nc):
    """Walrus CoreV3 codegen rejects instructions carrying more than one sync
    wait in several encodings (TPB_CTRL drains, S3_LW fused-weight matmuls).
    Move all waits of any multi-wait instruction onto preceding single-wait
    NOPs on the same engine — the engine executes them in order, so the
    semantics are identical."""
    for bb in nc.m.functions[0].blocks:
        new_insts = []
        for inst in bb.instructions:
            if (
                inst.sync_info
                and inst.sync_info.on_wait
                and len(inst.sync_info.on_wait) > 1
            ):
                waits = list(inst.sync_info.on_wait)
                for k, w in enumerate(waits[:-1]):
                    new_insts.append(
                        mybir.InstNoOp(
                            name=f"{inst.name}_wsplit{k}",
                            engine=inst.engine,
                            ins=[],
                            outs=[],
                            sync_info=mybir.SyncInfo(on_wait=[w], on_update=[]),
                        )
                    )
                inst.sync_info.on_wait = [waits[-1]]
            new_insts.append(inst)
        bb.instructions[:] = new_insts


def build_attention(NQ=4096, NK=4096, D=512, split_drains=True, repeat3=1):
    assert NQ % 512 == 0 and NK % 512 == 0 and D == 512
    DC = D // P          # 4 contraction chunks
    EC = D // P          # 4 output-feature chunks
    N_QC = NQ // 512     # q-chunks of 512 queries
    N_MC = NK // 512     # m-chunks of 512 keys
    N_MT = NK // P       # m-tiles of 128 keys
    SCALE = 1.0 / float(np.sqrt(D))

    nc = bass.Bass("TRN2", target_bir_lowering=False, debug=False,
                   num_devices=N_CORES)

    x_d = nc.dram_tensor("x", [NQ, D], F32, kind="ExternalInput").ap()
    ctx_d = nc.dram_tensor("context", [NK, D], F32, kind="ExternalInput").ap()
    wq_d = nc.dram_tensor("Wq", [D, D], F32, kind="ExternalInput").ap()
    bq_d = nc.dram_tensor("bq", [D], F32, kind="ExternalInput").ap()
    wk_d = nc.dram_tensor("Wk", [D, D], F32, kind="ExternalInput").ap()
    bk_d = nc.dram_tensor("bk", [D], F32, kind="ExternalInput").ap()
    wv_d = nc.dram_tensor("Wv", [D, D], F32, kind="ExternalInput").ap()
    bv_d = nc.dram_tensor("bv", [D], F32, kind="ExternalInput").ap()
    out_d = nc.dram_tensor("out", [NQ, D], F32, kind="ExternalOutput").ap()

    def r(ap):
        return ap.bitcast(F32R)

    with tile.TileContext(nc) as tc:
        with (
            tc.tile_pool(name="consts", bufs=1) as consts,
            tc.tile_pool(name="persist", bufs=1) as persist,
            tc.tile_pool(name="pnat", bufs=1) as pnat,
            tc.tile_pool(name="pct", bufs=2) as pct,
        ):
            ident = consts.tile([P, P], F32)
            make_identity(nc, ident)
            bq_sb = consts.tile([P, EC], F32)
            nc.sync.dma_start(out=bq_sb, in_=bq_d.rearrange("(c p) -> p c", p=P))
            bk_sb = consts.tile([P, EC], F32)
            nc.sync.dma_start(out=bk_sb, in_=bk_d.rearrange("(c p) -> p c", p=P))
            bv_bcast = consts.tile([P, D], F32)
            nc.gpsimd.dma_start(
                out=bv_bcast,
                in_=bass.AP(tensor=bv_d.tensor, offset=bv_d.offset,
                            ap=[[0, P], *bv_d.ap]),
            )

            KT_sb = persist.tile([P, EC, NK], F32R)     # K^T: [e-part, ec, m]
            V_sb = persist.tile([P, N_MT, D], F32R)     # V:   [m-part, mt, e]
            WqT_sb = persist.tile([P, DC, D], F32R)     # Wq^T: [d-part, dc, e]

            rep = (tc.For_i(0, repeat3, 1) if repeat3 > 1
                   else contextlib.nullcontext())
            with rep:
             with (
                 tc.tile_pool(name="wkv", bufs=1) as wkv,
                 tc.tile_pool(name="pt2", bufs=2, space="PSUM") as pt2,
                 tc.tile_pool(name="pk2", bufs=2, space="PSUM") as pk2,
                 tc.tile_pool(name="pv2", bufs=2, space="PSUM") as pv2,
             ):
                 WkT_sb = wkv.tile([P, DC, D], F32R)
                 WvT_sb = wkv.tile([P, DC, D], F32R)

                 # ---- Phase 1: load + transpose the three weight matrices ----
                 # Half-weight granularity so DMA overlaps transposes.
                 with tc.tile_pool(name="wnat", bufs=2) as wnat:
                     for w_dram, wT in ((wq_d, WqT_sb), (wk_d, WkT_sb),
                                        (wv_d, WvT_sb)):
                         for h in range(2):
                             w_nat = wnat.tile([P, 2, D], F32, tag="wnat")
                             nc.sync.dma_start(
                                 out=w_nat,
                                 in_=w_dram[h * 2 * P:(h + 1) * 2 * P, :]
                                 .rearrange("(c p) d -> p c d", p=P))
                             for e2 in range(2):
                                 ec = h * 2 + e2
                                 p_t = pt2.tile([P, 512], F32, tag="pt2")
                                 for dc in range(DC):
                                     nc.tensor.transpose(
                                         p_t[:, dc * P:(dc + 1) * P],
                                         w_nat[:, e2, dc * P:(dc + 1) * P], ident)
                                 nc.vector.tensor_copy(
                                     wT[:, 0:DC, ec * P:(ec + 1) * P],
                                     p_t.rearrange("p (c n) -> p c n", c=DC))

                 # ---- Phase 2: K^T and V projections, per 512-key chunk ----
                 if True:
                     for mc in range(N_MC):
                         c_nat = pnat.tile([P, 4, D], F32, tag="cnat")
                         nc.sync.dma_start(
                             out=c_nat,
                             in_=ctx_d[mc * 512:(mc + 1) * 512, :]
                             .rearrange("(j p) d -> p j d", p=P))
                         cT = pct.tile([P, DC, 512], F32R, tag="cT")
                         for jt in range(4):
                             p_t = pt2.tile([P, 512], F32, tag="pt2")
                             for dc in range(DC):
                                 nc.tensor.transpose(
                                     p_t[:, dc * P:(dc + 1) * P],
                                     c_nat[:, jt, dc * P:(dc + 1) * P], ident)
                             nc.vector.tensor_copy(
                                 cT[:, 0:DC, jt * P:(jt + 1) * P],
                                 p_t.rearrange("p (c n) -> p c n", c=DC))
                         # K^T[:, mc chunk] = Wk @ ctx^T  (+bk on evacuation)
                         for ec in range(EC):
                             p_k = pk2.tile([P, 512], F32, tag="pk")
                             for dc in range(DC):
                                 nc.tensor.matmul(
                                     p_k,
                                     WkT_sb[:, dc, ec * P:(ec + 1) * P],
                                     cT[:, dc, :],
                                     start=(dc == 0), stop=(dc == DC - 1))
                             nc.scalar.activation(
                                 KT_sb[:, ec, mc * 512:(mc + 1) * 512], p_k,
                                 mybir.ActivationFunctionType.Identity,
                                 bias=bk_sb[:, ec:ec + 1], scale=1.0)
                         # V rows (no bias)
                         for jt in range(4):
                             p_v = pv2.tile([P, D], F32, tag="pv")
                             for dc in range(DC):
                                 nc.tensor.matmul(
                                     p_v,
                                     cT[:, dc, jt * P:(jt + 1) * P],
                                     WvT_sb[:, dc, :],
                                     start=(dc == 0), stop=(dc == DC - 1))
                             nc.vector.tensor_copy(V_sb[:, mc * 4 + jt, :], p_v)

             # ---- Phase 3: attention, per 512-query chunk ----
             with (
                 tc.tile_pool(name="p3q", bufs=2) as p3q,
                 tc.tile_pool(name="p3e", bufs=2) as p3e,
                 tc.tile_pool(name="p3o", bufs=2) as p3o,
                 tc.tile_pool(name="p3r", bufs=2) as p3r,
                 tc.tile_pool(name="pacc", bufs=2) as pacc,
                 tc.tile_pool(name="ptq", bufs=2, space="PSUM") as ptq,
                 tc.tile_pool(name="ps", bufs=2, space="PSUM") as ps,
                 tc.tile_pool(name="po", bufs=4, space="PSUM") as po,
             ):
                 for qc in range(N_QC):
                     prio = (tc.high_priority(offset=360) if qc > 0
                             else contextlib.nullcontext())
                     with prio:
                         x_nat = pnat.tile([P, 4, D], F32, tag="cnat",
                                           name="x_nat")
                         nc.sync.dma_start(
                             out=x_nat,
                             in_=x_d[qc * 512:(qc + 1) * 512, :]
                             .rearrange("(j p) d -> p j d", p=P))
                         xT = pct.tile([P, DC, 512], F32R, tag="cT", name="xT")
                         for jt in range(4):
                             p_t = ptq.tile([P, 512], F32, tag="ptq")
                             for dc in range(DC):
                                 nc.tensor.transpose(
                                     p_t[:, dc * P:(dc + 1) * P],
                                     x_nat[:, jt, dc * P:(dc + 1) * P], ident)
                             nc.vector.tensor_copy(
                                 xT[:, 0:DC, jt * P:(jt + 1) * P],
                                 p_t.rearrange("p (c n) -> p c n", c=DC))
                         # Q^T chunk (+bq on evacuation)
                         QT = p3q.tile([P, EC, 512], F32R, tag="QT")
                         for ec in range(EC):
                             p_q = ptq.tile([P, 512], F32, tag="ptq")
                             for dc in range(DC):
                                 nc.tensor.matmul(
                                     p_q,
                                     WqT_sb[:, dc, ec * P:(ec + 1) * P],
                                     xT[:, dc, :],
                                     start=(dc == 0), stop=(dc == DC - 1))
                             nc.scalar.activation(
                                 QT[:, ec, :], p_q,
                                 mybir.ActivationFunctionType.Identity,
                                 bias=bq_sb[:, ec:ec + 1], scale=1.0)

                     p_o = [po.tile([P, D], F32, tag="po", name=f"po{i}")
                            for i in range(4)]
                     eacc = pacc.tile([P, 512], F32, tag="eacc")

                     for mt in range(N_MT):
                         p_s = ps.tile([P, 512], F32, tag="ps")
                         for ec in range(EC):
                             nc.tensor.matmul(
                                 p_s,
                                 KT_sb[:, ec, mt * P:(mt + 1) * P],
                                 QT[:, ec, :],
                                 start=(ec == 0), stop=(ec == EC - 1))
                         ET = p3e.tile([P, 512], F32R, tag="ET")
                         nc.scalar.activation(
                             ET, p_s, mybir.ActivationFunctionType.Exp,
                             bias=0.0, scale=SCALE)
                         if mt == 0:
                             nc.gpsimd.tensor_copy(eacc, ET.bitcast(F32))
                         else:
                             nc.gpsimd.tensor_add(eacc, eacc, ET.bitcast(F32))
                         for nt in range(4):
                             nc.tensor.matmul(
                                 p_o[nt], ET[:, nt * P:(nt + 1) * P],
                                 V_sb[:, mt, :],
                                 start=(mt == 0), stop=(mt == N_MT - 1))

                     # rowsum over m: transpose eacc on PE, free-dim reduce
                     # on DVE -> rs_T[p, c] = sum_m E[m, 128c+p]
                     accT = ptq.tile([P, 4, P], F32, tag="ptq", name="accT")
                     for c in range(4):
                         nc.tensor.transpose(
                             accT[:, c, :], eacc[:, c * P:(c + 1) * P], ident)
                     rs_T = p3r.tile([P, 4], F32, tag="rsT")
                     nc.vector.tensor_reduce(
                         rs_T, accT, axis=mybir.AxisListType.X,
                         op=mybir.AluOpType.add)
                     rinv = p3r.tile([P, 4], F32, tag="rinv")
                     nc.vector.reciprocal(rinv, rs_T)
                     for nt in range(4):
                         o_sb = p3o.tile([P, D], F32, tag="osb")
                         nc.vector.scalar_tensor_tensor(
                             out=o_sb, in0=p_o[nt], scalar=rinv[:, nt:nt + 1],
                             in1=bv_bcast,
                             op0=mybir.AluOpType.mult, op1=mybir.AluOpType.add)
                         nc.sync.dma_start(
                             out=out_d[qc * 512 + nt * P:
                                       qc * 512 + (nt + 1) * P, :],
                             in_=o_sb)

    if split_drains:
        _split_drain_waits(nc)
    return nc


_NC_CACHE = {}


def _get_nc(NQ, NK, D):
    key = (NQ, NK, D)
    if key not in _NC_CACHE:
        _NC_CACHE[key] = build_attention(NQ, NK, D)
    return _NC_CACHE[key]


def kernel(x, context, Wq, bq, Wk, bk, Wv, bv):
    x = np.asarray(x, dtype=np.float32)
    context = np.asarray(context, dtype=np.float32)
    Wq = np.asarray(Wq, dtype=np.float32)
    bq = np.asarray(bq, dtype=np.float32)
    Wk = np.asarray(Wk, dtype=np.float32)
    bk = np.asarray(bk, dtype=np.float32)
    Wv = np.asarray(Wv, dtype=np.float32)
    bv = np.asarray(bv, dtype=np.float32)

    B, NQ, D = x.shape
    NK = context.shape[1]
    assert B == N_CORES, f"expected batch {N_CORES}, got {B}"

    nc = _get_nc(NQ, NK, D)
    in_maps = [
        {
            "x": np.ascontiguousarray(x[b]),
            "context": np.ascontiguousarray(context[b]),
            "Wq": Wq, "bq": bq, "Wk": Wk, "bk": bk, "Wv": Wv, "bv": bv,
        }
        for b in range(B)
    ]
    res = run_bass_kernel_spmd(nc, in_maps, list(range(N_CORES)))
    return np.stack([res.results[b]["out"] for b in range(B)])

